# revision 6
# baseline (speedup 1.0000x reference)
"""CoAttention (Lorentz/hyperbolic co-attention) Trainium2 kernel.

Data-parallel over batch: 128 batches -> 16 per NeuronCore x 8 cores.
Per batch (Ns=Nc=512, D=101, KD=129):
  - three LorentzLinears fused into one matmul vs a combined [102,359] weight
  - affinity C = <comment, s~l>_L via PE (time-flip folded into s~l)
  - E = exp(2(C - m)) with runtime global max m (softmax denominators cancel
    inside Lorentz centroids, so no row sums are needed)
  - centroids + Lorentz addition in closed form:
      x (+) mu = [mu0*tx + d, (mu0 + d/(1+tx))*sx + mus],  d = <sx, mus>
    (cosh/sinh/arccosh all cancel), then LorentzAct tanh + reprojection
  - output attention a_s/a_c = true softmax over H logits
"""

import numpy as np

import concourse.bass as bass
import concourse.bacc as bacc
import concourse.mybir as mybir
import concourse.tile as tile
import concourse.masks as masks
import concourse.bass_isa as bass_isa
from concourse import bass_utils

F32 = mybir.dt.float32
AF = mybir.ActivationFunctionType
OP = mybir.AluOpType
AX = mybir.AxisListType

B_TOT = 128
NCORES = 8
BPC = B_TOT // NCORES     # batches per core
NT = 512                  # tokens (Ns = Nc)
TK = 4                    # token tiles of 128
D = 101                   # sentence/comment dim
DS = D - 1
KD = 129                  # lorentz-linear out dim (W_s/W_c)
KS = KD - 1
LW = 3 + DS + KS + KS     # 359 combined linear out cols
EPS = 1e-8

_CACHE = {}


def _bc(ap_slice, n):
    """Broadcast the last (count==1) dim of an AP slice to n via zero stride."""
    dims = [list(d) for d in ap_slice.ap]
    assert dims[-1][1] == 1
    dims[-1] = [0, n]
    return bass.AP(ap_slice.tensor, ap_slice.offset, dims)


def _bc_mid(col, n):
    """[128,1] AP -> [128, n, 1] via zero-stride middle dim."""
    dims = [list(col.ap[0]), [0, n], [0, 1]]
    return bass.AP(col.tensor, col.offset, dims)


def _build(scale_consts):
    """Build the 16-batch-per-core program. scale_consts = (e^s_l, e^s_s, e^s_c)."""
    A_l, A_s, A_c = scale_consts
    nc = bacc.Bacc("TRN2", target_bir_lowering=False, debug=False,
                   num_devices=NCORES)

    XS = nc.dram_tensor("xs", [BPC, NT, D], F32, kind="ExternalInput").ap()
    XC = nc.dram_tensor("xc", [BPC, NT, D], F32, kind="ExternalInput").ap()
    WCOMB = nc.dram_tensor("wcomb", [D + 1, LW], F32, kind="ExternalInput").ap()
    WHS = nc.dram_tensor("whs", [1, KD], F32, kind="ExternalInput").ap()
    WHC = nc.dram_tensor("whc", [1, KD], F32, kind="ExternalInput").ap()
    O_CO = nc.dram_tensor("o_co", [BPC, 2 * DS + 1], F32, kind="ExternalOutput").ap()
    O_AS = nc.dram_tensor("o_as", [BPC, NT], F32, kind="ExternalOutput").ap()
    O_AC = nc.dram_tensor("o_ac", [BPC, NT], F32, kind="ExternalOutput").ap()

    with tile.TileContext(nc) as tc:
        with (
            tc.tile_pool(name="const", bufs=1) as cpool,
            tc.tile_pool(name="w", bufs=2) as w,
            tc.tile_pool(name="ps_trans", bufs=1, space="PSUM") as ps_trans,
            tc.tile_pool(name="ps_big", bufs=2, space="PSUM") as ps_big,
            tc.tile_pool(name="ps_avg", bufs=2, space="PSUM") as ps_avg,
        ):
            # ---- constants ----
            wcomb = cpool.tile([D + 1, LW], F32, tag="wcomb")
            nc.sync.dma_start(wcomb[:], WCOMB[:])
            ident = cpool.tile([128, 128], F32, tag="ident")
            masks.make_identity(nc, ident[:])
            whs1 = cpool.tile([1, KD], F32, tag="whs1")
            nc.sync.dma_start(whs1[:], WHS[:])
            whc1 = cpool.tile([1, KD], F32, tag="whc1")
            nc.sync.dma_start(whc1[:], WHC[:])
            whs_b = cpool.tile([128, KD], F32, tag="whs_b")
            nc.gpsimd.partition_broadcast(whs_b[:], whs1[:])
            whc_b = cpool.tile([128, KD], F32, tag="whc_b")
            nc.gpsimd.partition_broadcast(whc_b[:], whc1[:])

            # persistent finals accumulator in SBUF on partition 0:
            # [1, batch, (s|c), dim]
            finsb = cpool.tile([1, BPC, 2, 128], F32, tag="finsb")

            for b in range(BPC):
                # ---- load inputs ----
                xs = w.tile([128, TK, D + 1], F32, tag="xs")
                nc.sync.dma_start(xs[:, :, 0:D],
                                  XS[b].rearrange("(k p) d -> p k d", p=128))
                nc.vector.memset(xs[:, :, D:D + 1], 1.0)
                xc = w.tile([128, TK, D], F32, tag="xc")
                nc.sync.dma_start(xc[:], XC[b].rearrange("(k p) d -> p k d", p=128))

                # ---- transpose inputs (PE) ----
                pt = ps_trans.tile([128, 512], F32, tag="pstrans")
                for k in range(TK):
                    nc.tensor.transpose(pt[0:D + 1, k * 128:(k + 1) * 128],
                                        xs[:, k, :], ident[:])
                xsT = w.tile([D + 1, 512], F32, tag="xsT")
                nc.scalar.copy(xsT[:], pt[0:D + 1, :])

                pt = ps_trans.tile([128, 512], F32, tag="pstrans")
                for k in range(TK):
                    nc.tensor.transpose(pt[0:D, k * 128:(k + 1) * 128],
                                        xc[:, k, :], ident[:])
                xcT = w.tile([D, 512], F32, tag="xcT")
                nc.scalar.copy(xcT[:], pt[0:D, :])

                # ---- lorentz linears (pairs of token tiles) ----
                slp = w.tile([128, TK, D], F32, tag="slp")   # s~l, time negated
                ws = w.tile([128, TK, KD], F32, tag="ws")
                wc = w.tile([128, TK, KD], F32, tag="wc")
                tt_ = w.tile([128, TK, 3], F32, tag="tt_")   # time per linear
                rws = w.tile([128, TK, 1], F32, tag="rws")   # 1/(1+t_ws)
                rwc = w.tile([128, TK, 1], F32, tag="rwc")
                scr3 = w.tile([128, TK, 3], F32, tag="scr3")
                scr3b = w.tile([128, TK, 3], F32, tag="scr3b")

                for pp in range(2):
                    sl2 = slice(pp * 2, pp * 2 + 2)
                    hp = ps_big.tile([128, 2, 512], F32, tag="big")
                    for kk in range(2):
                        k = pp * 2 + kk
                        nc.tensor.matmul(hp[:, kk, 0:LW],
                                         xsT[:, k * 128:(k + 1) * 128],
                                         wcomb[:], start=True, stop=True)
                    # sigmoid on the three packed time cols
                    sig = scr3
                    nc.scalar.activation(sig[:, sl2, :], hp[:, :, 0:3], AF.Sigmoid)
                    for li, Ax in enumerate((A_l, A_s, A_c)):
                        nc.vector.tensor_scalar(tt_[:, sl2, li:li + 1],
                                                sig[:, sl2, li:li + 1],
                                                float(Ax), 1.1, OP.mult, OP.add)
                    # sum of squares of each space block (squares on ACT)
                    junk = w.tile([128, 2, LW - 3], F32, tag="junk")
                    nc.scalar.activation(junk[:], hp[:, :, 3:LW], AF.Square)
                    ssq = scr3b
                    blocks = ((0, DS), (DS, DS + KS), (DS + KS, DS + 2 * KS))
                    for li, (c0, c1) in enumerate(blocks):
                        nc.vector.tensor_reduce(
                            ssq[:, sl2, li:li + 1], junk[:, :, c0:c1],
                            axis=AX.X, op=OP.add)
                    nc.vector.tensor_scalar_max(ssq[:, sl2, :], ssq[:, sl2, :], EPS)
                    rec = w.tile([128, 2, 3], F32, tag="rec")
                    nc.vector.reciprocal(rec[:], ssq[:, sl2, :])
                    t2 = w.tile([128, 2, 3], F32, tag="t2")
                    nc.vector.tensor_tensor(t2[:], tt_[:, sl2, :], tt_[:, sl2, :],
                                            OP.mult)
                    s2 = w.tile([128, 2, 3], F32, tag="s2")
                    nc.vector.scalar_tensor_tensor(s2[:], t2[:], 1.0, rec[:],
                                                   OP.subtract, OP.mult)
                    fac = w.tile([128, 2, 3], F32, tag="fac")
                    nc.scalar.activation(fac[:], s2[:], AF.Sqrt)
                    # outputs: space = fac * h_space ; time from tt_
                    nc.vector.tensor_tensor(slp[:, sl2, 1:D], hp[:, :, 3:3 + DS],
                                            _bc(fac[:, :, 0:1], DS), OP.mult)
                    nc.vector.tensor_tensor(ws[:, sl2, 1:KD],
                                            hp[:, :, 3 + DS:3 + DS + KS],
                                            _bc(fac[:, :, 1:2], KS), OP.mult)
                    nc.vector.tensor_tensor(wc[:, sl2, 1:KD],
                                            hp[:, :, 3 + DS + KS:LW],
                                            _bc(fac[:, :, 2:3], KS), OP.mult)
                    nc.vector.tensor_scalar_mul(slp[:, sl2, 0:1], tt_[:, sl2, 0:1],
                                                -1.0)
                    nc.vector.tensor_copy(ws[:, sl2, 0:1], tt_[:, sl2, 1:2])
                    nc.vector.tensor_copy(wc[:, sl2, 0:1], tt_[:, sl2, 2:3])
                    # rx = 1/(1+t) for ws and wc
                    tp1 = w.tile([128, 2, 2], F32, tag="tp1")
                    nc.vector.tensor_scalar_add(tp1[:], tt_[:, sl2, 1:3], 1.0)
                    nc.vector.reciprocal(rws[:, sl2, :], tp1[:, :, 0:1])
                    nc.vector.reciprocal(rwc[:, sl2, :], tp1[:, :, 1:2])

                # ---- s~l transpose ----
                pt = ps_trans.tile([128, 512], F32, tag="pstrans")
                for k in range(TK):
                    nc.tensor.transpose(pt[0:D, k * 128:(k + 1) * 128],
                                        slp[:, k, :], ident[:])
                slT = w.tile([D, 512], F32, tag="slT")
                nc.scalar.copy(slT[:], pt[0:D, :])

                # ---- affinity C (pairs) + row maxes ----
                mrow = w.tile([128, TK], F32, tag="mrow")
                cpairs = []
                for pp in range(2):
                    cp = ps_big.tile([128, 2, 512], F32, tag="big")
                    for kk in range(2):
                        j = pp * 2 + kk
                        nc.tensor.matmul(cp[:, kk, :],
                                         xcT[:, j * 128:(j + 1) * 128],
                                         slT[:], start=True, stop=True)
                    nc.vector.tensor_reduce(
                        mrow[:, pp * 2:pp * 2 + 2].rearrange("p (k o) -> p k o", o=1),
                        cp[:], axis=AX.X, op=OP.max)
                    cpairs.append(cp)
                m1 = w.tile([128, 1], F32, tag="m1")
                nc.vector.tensor_reduce(m1[:], mrow[:], axis=AX.X, op=OP.max)
                m2 = w.tile([128, 1], F32, tag="m2")
                nc.gpsimd.partition_all_reduce(m2[:], m1[:], channels=128,
                                               reduce_op=bass_isa.ReduceOp.max)
                bias_e = w.tile([128, 1], F32, tag="bias_e")
                nc.vector.tensor_scalar_mul(bias_e[:], m2[:], -2.0)

                # ---- E = exp(2C - 2m) ----
                E = w.tile([128, TK, 512], F32, tag="E")
                for pp in range(2):
                    nc.scalar.activation(E[:, pp * 2:pp * 2 + 2, :], cpairs[pp][:],
                                         AF.Exp, bias=bias_e[:], scale=2.0)

                # ---- ET (PE transpose of E) ----
                ET = w.tile([128, TK, 512], F32, tag="ET")
                for i in range(TK):
                    pte = ps_trans.tile([128, 512], F32, tag="pstrans")
                    for j in range(TK):
                        nc.tensor.transpose(pte[:, j * 128:(j + 1) * 128],
                                            E[:, j, i * 128:(i + 1) * 128], ident[:])
                    nc.vector.tensor_copy(ET[:, i, :], pte[:])

                # ---- centroid + lorentz addition + tanh for both sides ----
                def avg_side(lhsE, src, x_t, rx, tag):
                    Ut = w.tile([128, TK, KS], F32, tag="u" + tag)
                    H = w.tile([128, TK, KD], F32, tag="h" + tag)
                    for pq in range(2):
                        s2l = slice(pq * 2, pq * 2 + 2)
                        ap = ps_avg.tile([128, 2, KD], F32, tag="avgps")
                        for ii in range(2):
                            i = pq * 2 + ii
                            for kj in range(TK):
                                nc.tensor.matmul(
                                    ap[:, ii, :],
                                    lhsE[:, kj, i * 128:(i + 1) * 128],
                                    src[:, kj, :],
                                    start=(kj == 0), stop=(kj == TK - 1))
                        # per-i dots via stt with accumulator
                        jnk = w.tile([128, KD], F32, tag="jnk" + tag)
                        qs = w.tile([128, 2, 1], F32, tag="qs" + tag)
                        dts = w.tile([128, 2, 1], F32, tag="dts" + tag)
                        for ii in range(2):
                            i = pq * 2 + ii
                            nc.scalar.activation(
                                jnk[:], ap[:, ii, :], AF.Square,
                                accum_out=qs[:, ii, :])
                            nc.vector.scalar_tensor_tensor(
                                jnk[:, 0:KS], x_t[:, i, 1:KD], 1.0,
                                ap[:, ii, 1:KD], OP.mult, OP.mult,
                                accum_out=dts[:, ii, :])
                        # normalize by avg0 (positive scale, centroid-
                        # invariant) so fp32 never sees ~e^-80 magnitudes:
                        # invd = (1/sqrt(max(2 - qs/avg0^2, EPS))) / avg0
                        rb = w.tile([128, 2, 1], F32, tag="rb" + tag)
                        nc.vector.reciprocal(rb[:], ap[:, :, 0:1])
                        rb2 = w.tile([128, 2, 1], F32, tag="rb2" + tag)
                        nc.vector.tensor_tensor(rb2[:], rb[:], rb[:], OP.mult)
                        qq = w.tile([128, 2, 1], F32, tag="qq" + tag)
                        nc.vector.tensor_tensor(qq[:], qs[:], rb2[:], OP.mult)
                        nc.vector.tensor_scalar(qq[:], qq[:], -1.0, 2.0,
                                                OP.mult, OP.add)
                        nc.vector.tensor_scalar_max(qq[:], qq[:], EPS)
                        sd = w.tile([128, 2, 1], F32, tag="sd" + tag)
                        nc.scalar.activation(sd[:], qq[:], AF.Sqrt)
                        invd = w.tile([128, 2, 1], F32, tag="invd" + tag)
                        nc.vector.reciprocal(invd[:], sd[:])
                        nc.vector.tensor_tensor(invd[:], invd[:], rb[:], OP.mult)
                        # Atilde = dts*rx + avg0
                        aa = w.tile([128, 2, 1], F32, tag="aa" + tag)
                        nc.vector.tensor_tensor(aa[:], dts[:], rx[:, s2l, :], OP.mult)
                        nc.vector.tensor_tensor(aa[:], aa[:], ap[:, :, 0:1], OP.add)
                        # U = (Atilde*x_sp + avg_sp) * invd
                        for ii in range(2):
                            i = pq * 2 + ii
                            nc.vector.scalar_tensor_tensor(
                                Ut[:, i, :], x_t[:, i, 1:KD], aa[:, ii, :],
                                ap[:, ii, 1:KD], OP.mult, OP.add)
                        nc.gpsimd.tensor_tensor(Ut[:, s2l, :], Ut[:, s2l, :],
                                                _bc(invd[:], KS), OP.mult)
                    # tanh + reproject time
                    nc.scalar.activation(H[:, :, 1:KD], Ut[:], AF.Tanh)
                    th2 = w.tile([128, TK, 1], F32, tag="th2" + tag)
                    jnk2 = w.tile([128, KS], F32, tag="jnk2" + tag)
                    for i in range(TK):
                        nc.vector.scalar_tensor_tensor(
                            jnk2[:], H[:, i, 1:KD], 1.0, H[:, i, 1:KD],
                            OP.mult, OP.mult, accum_out=th2[:, i, :])
                    nc.scalar.activation(H[:, :, 0:1], th2[:], AF.Sqrt,
                                         bias=1.0, scale=1.0)
                    return H

                H_s = avg_side(E, wc, ws, rws, "s")
                H_c = avg_side(ET, ws, wc, rwc, "c")

                # ---- output attentions ----
                def attention(H, whx_b, tag):
                    Lr = w.tile([128, TK, 1], F32, tag="lr" + tag)
                    jnk3 = w.tile([128, KD], F32, tag="jnk3" + tag)
                    for i in range(TK):
                        nc.vector.scalar_tensor_tensor(
                            jnk3[:], H[:, i, :], 1.0, whx_b[:],
                            OP.mult, OP.mult, accum_out=Lr[:, i, :])
                    mx1 = w.tile([128, 1], F32, tag="mx1" + tag)
                    nc.vector.tensor_reduce(mx1[:], Lr[:].rearrange("p k o -> p (k o)"),
                                            axis=AX.X, op=OP.max)
                    mx2 = w.tile([128, 1], F32, tag="mx2" + tag)
                    nc.gpsimd.partition_all_reduce(mx2[:], mx1[:], channels=128,
                                                   reduce_op=bass_isa.ReduceOp.max)
                    nbias = w.tile([128, 1], F32, tag="nb" + tag)
                    nc.vector.tensor_scalar_mul(nbias[:], mx2[:], -2.0)
                    ex = w.tile([128, TK, 1], F32, tag="ex" + tag)
                    srow = w.tile([128, 1], F32, tag="srow" + tag)
                    nc.scalar.activation(ex[:], Lr[:], AF.Exp, bias=nbias[:],
                                         scale=2.0, accum_out=srow[:])
                    stot = w.tile([128, 1], F32, tag="stot" + tag)
                    nc.gpsimd.partition_all_reduce(stot[:], srow[:], channels=128,
                                                   reduce_op=bass_isa.ReduceOp.add)
                    rsum = w.tile([128, 1], F32, tag="rsum" + tag)
                    nc.vector.reciprocal(rsum[:], stot[:])
                    at = w.tile([128, TK, 1], F32, tag="at" + tag)
                    nc.vector.tensor_tensor(at[:], ex[:], _bc_mid(rsum[:], TK),
                                            OP.mult)
                    return at

                a_s = attention(H_s, whs_b, "s")
                a_c = attention(H_c, whc_b, "c")
                nc.sync.dma_start(O_AS[b].rearrange("(k p) -> p k", p=128),
                                  a_s[:].rearrange("p k o -> p (k o)"))
                nc.sync.dma_start(O_AC[b].rearrange("(k p) -> p k", p=128),
                                  a_c[:].rearrange("p k o -> p (k o)"))

                # ---- final centroids (M=1 matmuls), stash into SBUF row b ----
                fp = ps_avg.tile([1, 2, 128], F32, tag="avgps")
                for kj in range(TK):
                    nc.tensor.matmul(fp[:, 0, 0:D], a_s[:, kj, :],
                                     xs[:, kj, 0:D], start=(kj == 0),
                                     stop=(kj == TK - 1))
                for kj in range(TK):
                    nc.tensor.matmul(fp[:, 1, 0:D], a_c[:, kj, :],
                                     xc[:, kj, :], start=(kj == 0),
                                     stop=(kj == TK - 1))
                nc.vector.tensor_copy(finsb[:, b, :, 0:D], fp[:, :, 0:D])

            # ---- batched finals: normalize s,c and build co_sc ----
            finv = finsb[:, :, :, 0:D]                      # [1, BPC, 2, D]
            jnkf = cpool.tile([1, BPC, 2, D], F32, tag="jnkf")
            qf = cpool.tile([1, BPC, 2, 1], F32, tag="qf")
            nc.vector.tensor_tensor(jnkf[:], finv, finv, OP.mult)
            nc.vector.tensor_reduce(qf[:], jnkf[:], axis=AX.X, op=OP.add)
            p0f = cpool.tile([1, BPC, 2, 1], F32, tag="p0f")
            nc.vector.tensor_tensor(p0f[:], finv[:, :, :, 0:1],
                                    finv[:, :, :, 0:1], OP.mult)
            qf2 = cpool.tile([1, BPC, 2, 1], F32, tag="qf2")
            nc.vector.scalar_tensor_tensor(qf2[:], p0f[:], 2.0, qf[:],
                                           OP.mult, OP.subtract)
            nc.vector.tensor_scalar_max(qf2[:], qf2[:], EPS)
            sdf = cpool.tile([1, BPC, 2, 1], F32, tag="sdf")
            nc.scalar.activation(sdf[:], qf2[:], AF.Sqrt)
            invf = cpool.tile([1, BPC, 2, 1], F32, tag="invf")
            nc.vector.reciprocal(invf[:], sdf[:])
            co = cpool.tile([1, BPC, 2 * DS + 1], F32, tag="co")
            nc.vector.tensor_tensor(co[:, :, 1:DS + 1], finsb[:, :, 0, 1:D],
                                    _bc(invf[:, :, 0, 0:1], DS), OP.mult)
            nc.vector.tensor_tensor(co[:, :, DS + 1:2 * DS + 1],
                                    finsb[:, :, 1, 1:D],
                                    _bc(invf[:, :, 1, 0:1], DS), OP.mult)
            jnkg = cpool.tile([1, BPC, 2 * DS], F32, tag="jnkg")
            ssf = cpool.tile([1, BPC, 1], F32, tag="ssf")
            nc.vector.tensor_tensor(jnkg[:], co[:, :, 1:2 * DS + 1],
                                    co[:, :, 1:2 * DS + 1], OP.mult)
            nc.vector.tensor_reduce(ssf[:], jnkg[:], axis=AX.X, op=OP.add)
            nc.scalar.activation(co[:, :, 0:1], ssf[:], AF.Sqrt,
                                 bias=1.0, scale=1.0)
            nc.sync.dma_start(O_CO[:], co[:])

    nc.compile()
    return nc


def kernel(sentence_rep, comment_rep, W_l, b_l, s_l, W_s, b_s, s_s, W_c, b_c,
           s_c, w_hs, w_hc):
    sentence_rep = np.ascontiguousarray(sentence_rep, dtype=np.float32)
    comment_rep = np.ascontiguousarray(comment_rep, dtype=np.float32)

    scale_consts = (float(np.exp(s_l)), float(np.exp(s_s)), float(np.exp(s_c)))
    key = ("v1", scale_consts)
    if key not in _CACHE:
        _CACHE[key] = _build(scale_consts)
    nc = _CACHE[key]

    # combined linear weights: rows 0:101 inputs, row 101 bias;
    # cols [t_l, t_s, t_c, sp_l(100), sp_s(128), sp_c(128)]
    wcomb = np.zeros((D + 1, LW), dtype=np.float32)
    wcomb[0:D, 0] = W_l[0, :]
    wcomb[0:D, 1] = W_s[0, :]
    wcomb[0:D, 2] = W_c[0, :]
    wcomb[D, 0], wcomb[D, 1], wcomb[D, 2] = b_l[0], b_s[0], b_c[0]
    wcomb[0:D, 3:3 + DS] = W_l[1:, :].T
    wcomb[D, 3:3 + DS] = b_l[1:]
    wcomb[0:D, 3 + DS:3 + DS + KS] = W_s[1:, :].T
    wcomb[D, 3 + DS:3 + DS + KS] = b_s[1:]
    wcomb[0:D, 3 + DS + KS:LW] = W_c[1:, :].T
    wcomb[D, 3 + DS + KS:LW] = b_c[1:]

    whs_f = np.ascontiguousarray(w_hs, dtype=np.float32).reshape(1, KD).copy()
    whs_f[0, 0] *= -1.0
    whc_f = np.ascontiguousarray(w_hc, dtype=np.float32).reshape(1, KD).copy()
    whc_f[0, 0] *= -1.0

    in_maps = []
    for c in range(NCORES):
        sl_b = slice(c * BPC, (c + 1) * BPC)
        in_maps.append({
            "xs": sentence_rep[sl_b],
            "xc": comment_rep[sl_b],
            "wcomb": wcomb,
            "whs": whs_f,
            "whc": whc_f,
        })

    res = bass_utils.run_bass_kernel_spmd(nc, in_maps, core_ids=list(range(NCORES)))

    co = np.empty((B_TOT, 2 * DS + 1), dtype=np.float32)
    a_s = np.empty((B_TOT, 1, NT), dtype=np.float32)
    a_c = np.empty((B_TOT, 1, NT), dtype=np.float32)
    for c in range(NCORES):
        r = res.results[c]
        co[c * BPC:(c + 1) * BPC] = r["o_co"]
        a_s[c * BPC:(c + 1) * BPC, 0, :] = r["o_as"]
        a_c[c * BPC:(c + 1) * BPC, 0, :] = r["o_ac"]
    return co, a_s, a_c


# revision 8
# speedup vs baseline: 1.0382x; 1.0382x over previous
"""CoAttention (Lorentz/hyperbolic co-attention) Trainium2 kernel.

Data-parallel over batch: 128 batches -> 16 per NeuronCore x 8 cores.
Per batch (Ns=Nc=512, D=101, KD=129):
  - three LorentzLinears fused into one matmul vs a combined [102,359] weight
  - affinity C = <comment, s~l>_L via PE (time-flip folded into s~l)
  - E = exp(2(C - m)) with runtime global max m (softmax denominators cancel
    inside Lorentz centroids, so no row sums are needed)
  - centroids + Lorentz addition in closed form:
      x (+) mu = [mu0*tx + d, (mu0 + d/(1+tx))*sx + mus],  d = <sx, mus>
    (cosh/sinh/arccosh all cancel), then LorentzAct tanh + reprojection
  - output attention a_s/a_c = true softmax over H logits
"""

import numpy as np

import concourse.bass as bass
import concourse.bacc as bacc
import concourse.mybir as mybir
import concourse.tile as tile
import concourse.masks as masks
import concourse.bass_isa as bass_isa
from concourse import bass_utils

F32 = mybir.dt.float32
AF = mybir.ActivationFunctionType
OP = mybir.AluOpType
AX = mybir.AxisListType

B_TOT = 128
NCORES = 8
BPC = B_TOT // NCORES     # batches per core
NT = 512                  # tokens (Ns = Nc)
TK = 4                    # token tiles of 128
D = 101                   # sentence/comment dim
DS = D - 1
KD = 129                  # lorentz-linear out dim (W_s/W_c)
KS = KD - 1
LW = 3 + DS + KS + KS     # 359 combined linear out cols
EPS = 1e-8

_CACHE = {}


def _bc(ap_slice, n):
    """Broadcast the last (count==1) dim of an AP slice to n via zero stride."""
    dims = [list(d) for d in ap_slice.ap]
    assert dims[-1][1] == 1
    dims[-1] = [0, n]
    return bass.AP(ap_slice.tensor, ap_slice.offset, dims)


def _bc_mid(col, n):
    """[128,1] AP -> [128, n, 1] via zero-stride middle dim."""
    dims = [list(col.ap[0]), [0, n], [0, 1]]
    return bass.AP(col.tensor, col.offset, dims)


def _build(scale_consts, debug=False):
    """Build the 16-batch-per-core program. scale_consts = (e^s_l, e^s_s, e^s_c)."""
    A_l, A_s, A_c = scale_consts
    nc = bacc.Bacc("TRN2", target_bir_lowering=False, debug=False,
                   num_devices=NCORES)

    XS = nc.dram_tensor("xs", [BPC, NT, D], F32, kind="ExternalInput").ap()
    XC = nc.dram_tensor("xc", [BPC, NT, D], F32, kind="ExternalInput").ap()
    WCS = nc.dram_tensor("wcs", [D + 1, 2 + DS + KS], F32, kind="ExternalInput").ap()
    WCC = nc.dram_tensor("wcc", [D + 1, KD], F32, kind="ExternalInput").ap()
    WHS = nc.dram_tensor("whs", [1, KD], F32, kind="ExternalInput").ap()
    WHC = nc.dram_tensor("whc", [1, KD], F32, kind="ExternalInput").ap()
    O_CO = nc.dram_tensor("o_co", [BPC, 2 * DS + 1], F32, kind="ExternalOutput").ap()
    O_AS = nc.dram_tensor("o_as", [BPC, NT], F32, kind="ExternalOutput").ap()
    O_AC = nc.dram_tensor("o_ac", [BPC, NT], F32, kind="ExternalOutput").ap()
    dbg = {}
    if debug:
        for nm, shp in (("slp", [128, TK, D]), ("ws", [128, TK, KD]),
                        ("wc", [128, TK, KD]), ("m2", [128, 1]),
                        ("E", [128, TK, 512]), ("ET", [128, TK, 512]),
                        ("hs", [128, TK, KD]), ("hc", [128, TK, KD]),
                        ("xsT", [D + 1, 512]), ("slT", [D, 512])):
            dbg[nm] = nc.dram_tensor("dbg_" + nm, shp, F32,
                                     kind="ExternalOutput").ap()

    with tile.TileContext(nc) as tc:
        with (
            tc.tile_pool(name="const", bufs=1) as cpool,
            tc.tile_pool(name="w", bufs=2) as w,
            tc.tile_pool(name="ps_trans", bufs=1, space="PSUM") as ps_trans,
            tc.tile_pool(name="ps_big", bufs=2, space="PSUM") as ps_big,
            tc.tile_pool(name="ps_avg", bufs=2, space="PSUM") as ps_avg,
        ):
            # ---- constants ----
            wcs = cpool.tile([D + 1, 2 + DS + KS], F32, tag="wcs")
            nc.sync.dma_start(wcs[:], WCS[:])
            wcc = cpool.tile([D + 1, KD], F32, tag="wcc")
            nc.sync.dma_start(wcc[:], WCC[:])
            ident = cpool.tile([128, 128], F32, tag="ident")
            masks.make_identity(nc, ident[:])
            whs1 = cpool.tile([1, KD], F32, tag="whs1")
            nc.sync.dma_start(whs1[:], WHS[:])
            whc1 = cpool.tile([1, KD], F32, tag="whc1")
            nc.sync.dma_start(whc1[:], WHC[:])
            whs_b = cpool.tile([128, KD], F32, tag="whs_b")
            nc.gpsimd.partition_broadcast(whs_b[:], whs1[:])
            whc_b = cpool.tile([128, KD], F32, tag="whc_b")
            nc.gpsimd.partition_broadcast(whc_b[:], whc1[:])

            # persistent finals accumulator in SBUF on partition 0:
            # [1, batch, (s|c), dim]
            finsb = cpool.tile([1, BPC, 2, 128], F32, tag="finsb")

            for b in range(BPC):
                # ---- load inputs ----
                xs = w.tile([128, TK, D + 1], F32, tag="xs")
                nc.sync.dma_start(xs[:, :, 0:D],
                                  XS[b].rearrange("(k p) d -> p k d", p=128))
                nc.vector.memset(xs[:, :, D:D + 1], 1.0)
                xc = w.tile([128, TK, D + 1], F32, tag="xc")
                nc.sync.dma_start(xc[:, :, 0:D],
                                  XC[b].rearrange("(k p) d -> p k d", p=128))
                nc.vector.memset(xc[:, :, D:D + 1], 1.0)

                # ---- transpose inputs (PE) ----
                pt = ps_trans.tile([128, 512], F32, tag="pstrans")
                for k in range(TK):
                    nc.tensor.transpose(pt[0:D + 1, k * 128:(k + 1) * 128],
                                        xs[:, k, :], ident[:])
                xsT = w.tile([D + 1, 512], F32, tag="xsT")
                nc.scalar.copy(xsT[:], pt[0:D + 1, :])

                pt = ps_trans.tile([128, 512], F32, tag="pstrans")
                for k in range(TK):
                    nc.tensor.transpose(pt[0:D + 1, k * 128:(k + 1) * 128],
                                        xc[:, k, :], ident[:])
                xcT = w.tile([D + 1, 512], F32, tag="xcT")
                nc.scalar.copy(xcT[:], pt[0:D + 1, :])

                # ---- lorentz linears (pairs of token tiles) ----
                slp = w.tile([128, TK, D], F32, tag="slp")   # s~l, time negated
                ws = w.tile([128, TK, KD], F32, tag="ws")
                wc = w.tile([128, TK, KD], F32, tag="wc")
                tt_ = w.tile([128, TK, 3], F32, tag="tt_")   # time per linear
                rws = w.tile([128, TK, 1], F32, tag="rws")   # 1/(1+t_ws)
                rwc = w.tile([128, TK, 1], F32, tag="rwc")
                scr3 = w.tile([128, TK, 3], F32, tag="scr3")
                scr3b = w.tile([128, TK, 3], F32, tag="scr3b")

                for pp in range(2):
                    sl2 = slice(pp * 2, pp * 2 + 2)
                    hp = ps_big.tile([128, 2, 512], F32, tag="big")
                    NS_ = 2 + DS + KS    # 230
                    for kk in range(2):
                        k = pp * 2 + kk
                        nc.tensor.matmul(hp[:, kk, 0:NS_],
                                         xsT[:, k * 128:(k + 1) * 128],
                                         wcs[:], start=True, stop=True)
                        nc.tensor.matmul(hp[:, kk, NS_:NS_ + KD],
                                         xcT[:, k * 128:(k + 1) * 128],
                                         wcc[:], start=True, stop=True)
                    # sigmoid on the packed time cols (2 sentence + 1 comment)
                    sig = scr3
                    nc.scalar.activation(sig[:, sl2, 0:2], hp[:, :, 0:2], AF.Sigmoid)
                    nc.scalar.activation(sig[:, sl2, 2:3], hp[:, :, NS_:NS_ + 1],
                                         AF.Sigmoid)
                    for li, Ax in enumerate((A_l, A_s, A_c)):
                        nc.vector.tensor_scalar(tt_[:, sl2, li:li + 1],
                                                sig[:, sl2, li:li + 1],
                                                float(Ax), 1.1, OP.mult, OP.add)
                    # sum of squares of each space block (squares on ACT)
                    junk = w.tile([128, 2, LW - 3], F32, tag="junk")
                    nc.scalar.activation(junk[:, :, 0:DS + KS],
                                         hp[:, :, 2:NS_], AF.Square)
                    nc.scalar.activation(junk[:, :, DS + KS:],
                                         hp[:, :, NS_ + 1:NS_ + KD], AF.Square)
                    ssq = scr3b
                    blocks = ((0, DS), (DS, DS + KS), (DS + KS, DS + 2 * KS))
                    for li, (c0, c1) in enumerate(blocks):
                        nc.vector.tensor_reduce(
                            ssq[:, sl2, li:li + 1], junk[:, :, c0:c1],
                            axis=AX.X, op=OP.add)
                    nc.vector.tensor_scalar_max(ssq[:, sl2, :], ssq[:, sl2, :], EPS)
                    rec = w.tile([128, 2, 3], F32, tag="rec")
                    nc.vector.reciprocal(rec[:], ssq[:, sl2, :])
                    t2 = w.tile([128, 2, 3], F32, tag="t2")
                    nc.vector.tensor_tensor(t2[:], tt_[:, sl2, :], tt_[:, sl2, :],
                                            OP.mult)
                    s2 = w.tile([128, 2, 3], F32, tag="s2")
                    nc.vector.scalar_tensor_tensor(s2[:], t2[:], 1.0, rec[:],
                                                   OP.subtract, OP.mult)
                    fac = w.tile([128, 2, 3], F32, tag="fac")
                    nc.scalar.activation(fac[:], s2[:], AF.Sqrt)
                    # outputs: space = fac * h_space ; time from tt_
                    nc.vector.tensor_tensor(slp[:, sl2, 1:D], hp[:, :, 2:2 + DS],
                                            _bc(fac[:, :, 0:1], DS), OP.mult)
                    nc.vector.tensor_tensor(ws[:, sl2, 1:KD],
                                            hp[:, :, 2 + DS:NS_],
                                            _bc(fac[:, :, 1:2], KS), OP.mult)
                    nc.vector.tensor_tensor(wc[:, sl2, 1:KD],
                                            hp[:, :, NS_ + 1:NS_ + KD],
                                            _bc(fac[:, :, 2:3], KS), OP.mult)
                    nc.vector.tensor_scalar_mul(slp[:, sl2, 0:1], tt_[:, sl2, 0:1],
                                                -1.0)
                    nc.vector.tensor_copy(ws[:, sl2, 0:1], tt_[:, sl2, 1:2])
                    nc.vector.tensor_copy(wc[:, sl2, 0:1], tt_[:, sl2, 2:3])
                    # rx = 1/(1+t) for ws and wc
                    tp1 = w.tile([128, 2, 2], F32, tag="tp1")
                    nc.vector.tensor_scalar_add(tp1[:], tt_[:, sl2, 1:3], 1.0)
                    nc.vector.reciprocal(rws[:, sl2, :], tp1[:, :, 0:1])
                    nc.vector.reciprocal(rwc[:, sl2, :], tp1[:, :, 1:2])

                # ---- s~l transpose ----
                pt = ps_trans.tile([128, 512], F32, tag="pstrans")
                for k in range(TK):
                    nc.tensor.transpose(pt[0:D, k * 128:(k + 1) * 128],
                                        slp[:, k, :], ident[:])
                slT = w.tile([D, 512], F32, tag="slT")
                nc.scalar.copy(slT[:], pt[0:D, :])
                if debug and b == 0:
                    nc.sync.dma_start(dbg["slp"], slp[:])
                    nc.sync.dma_start(dbg["ws"], ws[:])
                    nc.sync.dma_start(dbg["wc"], wc[:])
                    nc.sync.dma_start(dbg["xsT"], xsT[:])
                    nc.sync.dma_start(dbg["slT"], slT[:])

                # ---- affinity C (pairs) + row maxes ----
                mrow = w.tile([128, TK], F32, tag="mrow")
                cpairs = []
                for pp in range(2):
                    cp = ps_big.tile([128, 2, 512], F32, tag="big")
                    for kk in range(2):
                        j = pp * 2 + kk
                        nc.tensor.matmul(cp[:, kk, :],
                                         xcT[0:D, j * 128:(j + 1) * 128],
                                         slT[:], start=True, stop=True)
                    nc.vector.tensor_reduce(
                        mrow[:, pp * 2:pp * 2 + 2].rearrange("p (k o) -> p k o", o=1),
                        cp[:], axis=AX.X, op=OP.max)
                    cpairs.append(cp)
                m1 = w.tile([128, 1], F32, tag="m1")
                nc.vector.tensor_reduce(m1[:], mrow[:], axis=AX.X, op=OP.max)
                m2 = w.tile([128, 1], F32, tag="m2")
                nc.gpsimd.partition_all_reduce(m2[:], m1[:], channels=128,
                                               reduce_op=bass_isa.ReduceOp.max)
                bias_e = w.tile([128, 1], F32, tag="bias_e")
                nc.vector.tensor_scalar_mul(bias_e[:], m2[:], -2.0)

                # ---- E = exp(2C - 2m) ----
                E = w.tile([128, TK, 512], F32, tag="E")
                for pp in range(2):
                    nc.scalar.activation(E[:, pp * 2:pp * 2 + 2, :], cpairs[pp][:],
                                         AF.Exp, bias=bias_e[:], scale=2.0)

                if debug and b == 0:
                    nc.sync.dma_start(dbg["m2"], m2[:])
                    nc.sync.dma_start(dbg["E"], E[:])
                # ---- ET (PE transpose of E) ----
                ET = w.tile([128, TK, 512], F32, tag="ET")
                for i in range(TK):
                    pte = ps_trans.tile([128, 512], F32, tag="pstrans")
                    for j in range(TK):
                        nc.tensor.transpose(pte[:, j * 128:(j + 1) * 128],
                                            E[:, j, i * 128:(i + 1) * 128], ident[:])
                    nc.vector.tensor_copy(ET[:, i, :], pte[:])

                # ---- centroid + lorentz addition + tanh for both sides ----
                def avg_side(lhsE, src, x_t, rx, tag):
                    Ut = w.tile([128, TK, KS], F32, tag="u" + tag)
                    H = w.tile([128, TK, KD], F32, tag="h" + tag)
                    for pq in range(2):
                        s2l = slice(pq * 2, pq * 2 + 2)
                        ap = ps_avg.tile([128, 2, KD], F32, tag="avgps")
                        for ii in range(2):
                            i = pq * 2 + ii
                            for kj in range(TK):
                                nc.tensor.matmul(
                                    ap[:, ii, :],
                                    lhsE[:, kj, i * 128:(i + 1) * 128],
                                    src[:, kj, :],
                                    start=(kj == 0), stop=(kj == TK - 1))
                        # per-i dots via stt with accumulator
                        jnk = w.tile([128, KD], F32, tag="jnk" + tag)
                        qs = w.tile([128, 2, 1], F32, tag="qs" + tag)
                        dts = w.tile([128, 2, 1], F32, tag="dts" + tag)
                        for ii in range(2):
                            i = pq * 2 + ii
                            nc.scalar.activation(
                                jnk[:], ap[:, ii, :], AF.Square,
                                accum_out=qs[:, ii, :])
                            nc.vector.scalar_tensor_tensor(
                                jnk[:, 0:KS], x_t[:, i, 1:KD], 1.0,
                                ap[:, ii, 1:KD], OP.mult, OP.mult,
                                accum_out=dts[:, ii, :])
                        # normalize by avg0 (positive scale, centroid-
                        # invariant) so fp32 never sees ~e^-80 magnitudes:
                        # invd = (1/sqrt(max(2 - qs/avg0^2, EPS))) / avg0
                        rb = w.tile([128, 2, 1], F32, tag="rb" + tag)
                        nc.vector.reciprocal(rb[:], ap[:, :, 0:1])
                        rb2 = w.tile([128, 2, 1], F32, tag="rb2" + tag)
                        nc.vector.tensor_tensor(rb2[:], rb[:], rb[:], OP.mult)
                        qq = w.tile([128, 2, 1], F32, tag="qq" + tag)
                        nc.vector.tensor_tensor(qq[:], qs[:], rb2[:], OP.mult)
                        nc.vector.tensor_scalar(qq[:], qq[:], -1.0, 2.0,
                                                OP.mult, OP.add)
                        nc.vector.tensor_scalar_max(qq[:], qq[:], EPS)
                        sd = w.tile([128, 2, 1], F32, tag="sd" + tag)
                        nc.scalar.activation(sd[:], qq[:], AF.Sqrt)
                        invd = w.tile([128, 2, 1], F32, tag="invd" + tag)
                        nc.vector.reciprocal(invd[:], sd[:])
                        nc.vector.tensor_tensor(invd[:], invd[:], rb[:], OP.mult)
                        # Atilde = dts*rx + avg0
                        aa = w.tile([128, 2, 1], F32, tag="aa" + tag)
                        nc.vector.tensor_tensor(aa[:], dts[:], rx[:, s2l, :], OP.mult)
                        nc.vector.tensor_tensor(aa[:], aa[:], ap[:, :, 0:1], OP.add)
                        # U = (Atilde*x_sp + avg_sp) * invd
                        for ii in range(2):
                            i = pq * 2 + ii
                            nc.vector.scalar_tensor_tensor(
                                Ut[:, i, :], x_t[:, i, 1:KD], aa[:, ii, :],
                                ap[:, ii, 1:KD], OP.mult, OP.add)
                        nc.gpsimd.tensor_tensor(Ut[:, s2l, :], Ut[:, s2l, :],
                                                _bc(invd[:], KS), OP.mult)
                    # tanh + reproject time
                    nc.scalar.activation(H[:, :, 1:KD], Ut[:], AF.Tanh)
                    th2 = w.tile([128, TK, 1], F32, tag="th2" + tag)
                    jnk2 = w.tile([128, KS], F32, tag="jnk2" + tag)
                    for i in range(TK):
                        nc.vector.scalar_tensor_tensor(
                            jnk2[:], H[:, i, 1:KD], 1.0, H[:, i, 1:KD],
                            OP.mult, OP.mult, accum_out=th2[:, i, :])
                    nc.scalar.activation(H[:, :, 0:1], th2[:], AF.Sqrt,
                                         bias=1.0, scale=1.0)
                    return H

                H_s = avg_side(E, wc, ws, rws, "s")
                H_c = avg_side(ET, ws, wc, rwc, "c")
                if debug and b == 0:
                    nc.sync.dma_start(dbg["ET"], ET[:])
                    nc.sync.dma_start(dbg["hs"], H_s[:])
                    nc.sync.dma_start(dbg["hc"], H_c[:])

                # ---- output attentions ----
                def attention(H, whx_b, tag):
                    Lr = w.tile([128, TK, 1], F32, tag="lr" + tag)
                    jnk3 = w.tile([128, KD], F32, tag="jnk3" + tag)
                    for i in range(TK):
                        nc.vector.scalar_tensor_tensor(
                            jnk3[:], H[:, i, :], 1.0, whx_b[:],
                            OP.mult, OP.mult, accum_out=Lr[:, i, :])
                    mx1 = w.tile([128, 1], F32, tag="mx1" + tag)
                    nc.vector.tensor_reduce(mx1[:], Lr[:].rearrange("p k o -> p (k o)"),
                                            axis=AX.X, op=OP.max)
                    mx2 = w.tile([128, 1], F32, tag="mx2" + tag)
                    nc.gpsimd.partition_all_reduce(mx2[:], mx1[:], channels=128,
                                                   reduce_op=bass_isa.ReduceOp.max)
                    nbias = w.tile([128, 1], F32, tag="nb" + tag)
                    nc.vector.tensor_scalar_mul(nbias[:], mx2[:], -2.0)
                    ex = w.tile([128, TK, 1], F32, tag="ex" + tag)
                    srow = w.tile([128, 1], F32, tag="srow" + tag)
                    nc.scalar.activation(ex[:], Lr[:], AF.Exp, bias=nbias[:],
                                         scale=2.0, accum_out=srow[:])
                    stot = w.tile([128, 1], F32, tag="stot" + tag)
                    nc.gpsimd.partition_all_reduce(stot[:], srow[:], channels=128,
                                                   reduce_op=bass_isa.ReduceOp.add)
                    rsum = w.tile([128, 1], F32, tag="rsum" + tag)
                    nc.vector.reciprocal(rsum[:], stot[:])
                    at = w.tile([128, TK, 1], F32, tag="at" + tag)
                    nc.vector.tensor_tensor(at[:], ex[:], _bc_mid(rsum[:], TK),
                                            OP.mult)
                    return at

                a_s = attention(H_s, whs_b, "s")
                a_c = attention(H_c, whc_b, "c")
                nc.sync.dma_start(O_AS[b].rearrange("(k p) -> p k", p=128),
                                  a_s[:].rearrange("p k o -> p (k o)"))
                nc.sync.dma_start(O_AC[b].rearrange("(k p) -> p k", p=128),
                                  a_c[:].rearrange("p k o -> p (k o)"))

                # ---- final centroids (M=1 matmuls), stash into SBUF row b ----
                fp = ps_avg.tile([1, 2, 128], F32, tag="avgps")
                for kj in range(TK):
                    nc.tensor.matmul(fp[:, 0, 0:D], a_s[:, kj, :],
                                     xs[:, kj, 0:D], start=(kj == 0),
                                     stop=(kj == TK - 1))
                for kj in range(TK):
                    nc.tensor.matmul(fp[:, 1, 0:D], a_c[:, kj, :],
                                     xc[:, kj, 0:D], start=(kj == 0),
                                     stop=(kj == TK - 1))
                nc.vector.tensor_copy(finsb[:, b, :, 0:D], fp[:, :, 0:D])

            # ---- batched finals: normalize s,c and build co_sc ----
            finv = finsb[:, :, :, 0:D]                      # [1, BPC, 2, D]
            jnkf = cpool.tile([1, BPC, 2, D], F32, tag="jnkf")
            qf = cpool.tile([1, BPC, 2, 1], F32, tag="qf")
            nc.vector.tensor_tensor(jnkf[:], finv, finv, OP.mult)
            nc.vector.tensor_reduce(qf[:], jnkf[:], axis=AX.X, op=OP.add)
            p0f = cpool.tile([1, BPC, 2, 1], F32, tag="p0f")
            nc.vector.tensor_tensor(p0f[:], finv[:, :, :, 0:1],
                                    finv[:, :, :, 0:1], OP.mult)
            qf2 = cpool.tile([1, BPC, 2, 1], F32, tag="qf2")
            nc.vector.scalar_tensor_tensor(qf2[:], p0f[:], 2.0, qf[:],
                                           OP.mult, OP.subtract)
            nc.vector.tensor_scalar_max(qf2[:], qf2[:], EPS)
            sdf = cpool.tile([1, BPC, 2, 1], F32, tag="sdf")
            nc.scalar.activation(sdf[:], qf2[:], AF.Sqrt)
            invf = cpool.tile([1, BPC, 2, 1], F32, tag="invf")
            nc.vector.reciprocal(invf[:], sdf[:])
            co = cpool.tile([1, BPC, 2 * DS + 1], F32, tag="co")
            nc.vector.tensor_tensor(co[:, :, 1:DS + 1], finsb[:, :, 0, 1:D],
                                    _bc(invf[:, :, 0, 0:1], DS), OP.mult)
            nc.vector.tensor_tensor(co[:, :, DS + 1:2 * DS + 1],
                                    finsb[:, :, 1, 1:D],
                                    _bc(invf[:, :, 1, 0:1], DS), OP.mult)
            jnkg = cpool.tile([1, BPC, 2 * DS], F32, tag="jnkg")
            ssf = cpool.tile([1, BPC, 1], F32, tag="ssf")
            nc.vector.tensor_tensor(jnkg[:], co[:, :, 1:2 * DS + 1],
                                    co[:, :, 1:2 * DS + 1], OP.mult)
            nc.vector.tensor_reduce(ssf[:], jnkg[:], axis=AX.X, op=OP.add)
            nc.scalar.activation(co[:, :, 0:1], ssf[:], AF.Sqrt,
                                 bias=1.0, scale=1.0)
            nc.sync.dma_start(O_CO[:], co[:])

    nc.compile()
    return nc


def kernel(sentence_rep, comment_rep, W_l, b_l, s_l, W_s, b_s, s_s, W_c, b_c,
           s_c, w_hs, w_hc):
    sentence_rep = np.ascontiguousarray(sentence_rep, dtype=np.float32)
    comment_rep = np.ascontiguousarray(comment_rep, dtype=np.float32)

    import os
    debug = bool(os.environ.get("KERNEL_DEBUG"))
    scale_consts = (float(np.exp(s_l)), float(np.exp(s_s)), float(np.exp(s_c)))
    key = ("v1", scale_consts, debug)
    if key not in _CACHE:
        _CACHE[key] = _build(scale_consts, debug=debug)
    nc = _CACHE[key]

    # combined linear weights: rows 0:101 inputs, row 101 bias.
    # sentence side cols [t_l, t_s, sp_l(100), sp_s(128)]; comment side
    # cols [t_c, sp_c(128)]
    wcs = np.zeros((D + 1, 2 + DS + KS), dtype=np.float32)
    wcs[0:D, 0] = W_l[0, :]
    wcs[0:D, 1] = W_s[0, :]
    wcs[D, 0], wcs[D, 1] = b_l[0], b_s[0]
    wcs[0:D, 2:2 + DS] = W_l[1:, :].T
    wcs[D, 2:2 + DS] = b_l[1:]
    wcs[0:D, 2 + DS:] = W_s[1:, :].T
    wcs[D, 2 + DS:] = b_s[1:]
    wcc = np.zeros((D + 1, KD), dtype=np.float32)
    wcc[0:D, 0] = W_c[0, :]
    wcc[D, 0] = b_c[0]
    wcc[0:D, 1:] = W_c[1:, :].T
    wcc[D, 1:] = b_c[1:]

    whs_f = np.ascontiguousarray(w_hs, dtype=np.float32).reshape(1, KD).copy()
    whs_f[0, 0] *= -1.0
    whc_f = np.ascontiguousarray(w_hc, dtype=np.float32).reshape(1, KD).copy()
    whc_f[0, 0] *= -1.0

    in_maps = []
    for c in range(NCORES):
        sl_b = slice(c * BPC, (c + 1) * BPC)
        in_maps.append({
            "xs": sentence_rep[sl_b],
            "xc": comment_rep[sl_b],
            "wcs": wcs,
            "wcc": wcc,
            "whs": whs_f,
            "whc": whc_f,
        })

    res = bass_utils.run_bass_kernel_spmd(nc, in_maps, core_ids=list(range(NCORES)))

    co = np.empty((B_TOT, 2 * DS + 1), dtype=np.float32)
    a_s = np.empty((B_TOT, 1, NT), dtype=np.float32)
    a_c = np.empty((B_TOT, 1, NT), dtype=np.float32)
    for c in range(NCORES):
        r = res.results[c]
        co[c * BPC:(c + 1) * BPC] = r["o_co"]
        a_s[c * BPC:(c + 1) * BPC, 0, :] = r["o_as"]
        a_c[c * BPC:(c + 1) * BPC, 0, :] = r["o_ac"]
    if debug:
        kernel._dbg = {k: v for k, v in res.results[0].items()
                       if k.startswith("dbg_")}
    return co, a_s, a_c
